# revision 8
# baseline (speedup 1.0000x reference)
"""GQA forward on 8 Trainium2 NeuronCores.

Sharding: core c -> batch b=c//4, kv-head pair p=c%4 (kv heads {2p,2p+1},
q heads 8p..8p+7). Each core computes a partial [T,E] output (its heads'
contribution through Wo rows); host sums the 4 partials per batch + bo.

Device layout choices:
- Host passes x[b].T (bias-augmented, bf16) so no on-device transpose of x.
- Q/K/V projections computed in [t,d] layout; RoPE applied there with
  host-precomputed t-major cos/sin tables; Q,K then PE-transposed to [d,t].
- Scores computed as S^T = K Q^T (row-packed matmul pairs, K=64 each),
  exp on ScalarE (scale=1/8 folded in), causal handled by block skipping +
  one [128,128] mask multiply per diagonal block.
- A@V computed in O^T form: lhsT = [V | ones] (stationary), rhs = exp(S^T),
  giving [65, i] PSUM tiles where row 64 is the softmax denominator Z.
  Normalize via reciprocal + gpsimd partition_broadcast + fused DVE multiply
  straight into the yT buffer, which is already the lhsT layout needed for
  the output projection.
"""
import sys
import numpy as np

sys.path.insert(0, "/opt/trn_rl_repo")

import ml_dtypes

BF16 = ml_dtypes.bfloat16

B, T, E = 2, 2048, 2048
HQ, HKV = 32, 8
D = 64
NT = T // 128          # 16 t-chunks
KC = 17                # augmented contraction chunks (2048 + bias row -> 2176)
KAUG = KC * 128

_cache = {}


def _build_program():
    import concourse.bass as bass
    import concourse.tile as tile
    import concourse.mybir as mybir
    from concourse import bacc

    fp32 = mybir.dt.float32
    bf16 = mybir.dt.bfloat16
    MUL = mybir.AluOpType.mult
    ADD = mybir.AluOpType.add
    SUB = mybir.AluOpType.subtract
    EXP = mybir.ActivationFunctionType.Exp

    nc = bacc.Bacc("TRN2", target_bir_lowering=False, debug=False)

    xt_d = nc.dram_tensor("xt", [KAUG, T], bf16, kind="ExternalInput").ap()
    wq_d = nc.dram_tensor("wq", [KAUG, 512], bf16, kind="ExternalInput").ap()
    wk_d = nc.dram_tensor("wk", [KAUG, 128], bf16, kind="ExternalInput").ap()
    wv_d = nc.dram_tensor("wv", [KAUG, 128], bf16, kind="ExternalInput").ap()
    wo_d = nc.dram_tensor("wo", [512, T], bf16, kind="ExternalInput").ap()
    rope_d = nc.dram_tensor("rope", [T, 512], fp32, kind="ExternalInput").ap()
    mask_d = nc.dram_tensor("mask", [128, 128], bf16, kind="ExternalInput").ap()
    iden_d = nc.dram_tensor("iden", [128, 128], bf16, kind="ExternalInput").ap()
    out_d = nc.dram_tensor("out", [T, E], fp32, kind="ExternalOutput").ap()

    def hv(ap, H, off, w):
        # [128, H*64] -> [128, H, w] slice of each head's d-range [off, off+w)
        return ap.rearrange("p (h d) -> p h d", h=H)[:, :, off:off + w]

    with tile.TileContext(nc) as tc:
        with (
            tc.tile_pool(name="persist", bufs=1) as pp,
            tc.tile_pool(name="xpool", bufs=1) as xp,
            tc.tile_pool(name="wpool", bufs=1) as wp,
        ):
            iden = pp.tile([128, 128], bf16)
            nc.sync.dma_start(iden[:], iden_d[:])
            mask = pp.tile([128, 128], bf16)
            nc.sync.dma_start(mask[:], mask_d[:])

            xts = []
            wqs, wks, wvs = [], [], []
            for kc in range(KC):
                xt = xp.tile([128, T], bf16, tag=f"xt{kc}")
                nc.sync.dma_start(xt[:], xt_d[kc * 128:(kc + 1) * 128, :])
                xts.append(xt)
                wq = wp.tile([128, 512], bf16, tag=f"wq{kc}")
                nc.sync.dma_start(wq[:], wq_d[kc * 128:(kc + 1) * 128, :])
                wqs.append(wq)
                wk = wp.tile([128, 128], bf16, tag=f"wk{kc}")
                nc.sync.dma_start(wk[:], wk_d[kc * 128:(kc + 1) * 128, :])
                wks.append(wk)
                wv = wp.tile([128, 128], bf16, tag=f"wv{kc}")
                nc.sync.dma_start(wv[:], wv_d[kc * 128:(kc + 1) * 128, :])
                wvs.append(wv)

            # persistent intermediate buffers
            QT = [pp.tile([128, T], bf16, tag=f"QT{i}", name=f"QT{i}") for i in range(4)]
            KTd = [pp.tile([128, T], bf16, tag=f"KTd{i}", name=f"KTd{i}") for i in range(2)]
            Vs = [pp.tile([128, 130], bf16, tag=f"V{j}", name=f"V{j}") for j in range(NT)]
            yT = [pp.tile([128, T], bf16, tag=f"yT{i}", name=f"yT{i}") for i in range(4)]

            # ---------------- Phase 1: projections + rope + transposes --------
            with (
                tc.tile_pool(name="ps1", bufs=2, space="PSUM") as ps1,
                tc.tile_pool(name="pskv", bufs=2, space="PSUM") as pskv,
                tc.tile_pool(name="pst", bufs=2, space="PSUM") as pst,
                tc.tile_pool(name="rope", bufs=2) as rp,
                tc.tile_pool(name="tmp", bufs=4) as tp,
                tc.tile_pool(name="qk", bufs=2) as qkp,
            ):
                for t_i in range(NT):
                    ts = slice(t_i * 128, (t_i + 1) * 128)
                    psQ = ps1.tile([128, 512], fp32, tag="psQ")
                    psK = pskv.tile([128, 128], fp32, tag="psK")
                    psV = pskv.tile([128, 128], fp32, tag="psV")
                    for kc in range(KC):
                        st, sp = kc == 0, kc == KC - 1
                        lhs = xts[kc][:, ts]
                        nc.tensor.matmul(psQ[:], lhs, wqs[kc][:], start=st, stop=sp)
                        nc.tensor.matmul(psK[:], lhs, wks[kc][:], start=st, stop=sp)
                        nc.tensor.matmul(psV[:], lhs, wvs[kc][:], start=st, stop=sp)

                    rt = rp.tile([128, 512], fp32, tag="rt")
                    nc.sync.dma_start(rt[:], rope_d[ts, :])

                    # RoPE Q on DVE: y1' = y1*c - y2*s ; y2' = y2*c + y1*s
                    Qsb = qkp.tile([128, 512], bf16, tag="Qsb")
                    q1 = hv(psQ[:], 8, 0, 32)
                    q2 = hv(psQ[:], 8, 32, 32)
                    c8v = hv(rt[:, 0:256], 8, 0, 32)
                    s8v = hv(rt[:, 256:512], 8, 0, 32)
                    ta = tp.tile([128, 256], fp32, tag="ta")
                    tb = tp.tile([128, 256], fp32, tag="tb")
                    tav = hv(ta[:], 8, 0, 32)
                    tbv = hv(tb[:], 8, 0, 32)
                    nc.vector.tensor_tensor(out=tav, in0=q1, in1=c8v, op=MUL)
                    nc.vector.tensor_tensor(out=tbv, in0=q2, in1=s8v, op=MUL)
                    nc.vector.tensor_tensor(out=hv(Qsb[:], 8, 0, 32), in0=tav, in1=tbv, op=SUB)
                    tc_ = tp.tile([128, 256], fp32, tag="tc")
                    td_ = tp.tile([128, 256], fp32, tag="td")
                    tcv = hv(tc_[:], 8, 0, 32)
                    tdv = hv(td_[:], 8, 0, 32)
                    nc.vector.tensor_tensor(out=tcv, in0=q2, in1=c8v, op=MUL)
                    nc.vector.tensor_tensor(out=tdv, in0=q1, in1=s8v, op=MUL)
                    nc.vector.tensor_tensor(out=hv(Qsb[:], 8, 32, 32), in0=tcv, in1=tdv, op=ADD)

                    # RoPE K on GpSimd
                    Ksb = qkp.tile([128, 128], bf16, tag="Ksb")
                    k1 = hv(psK[:], 2, 0, 32)
                    k2 = hv(psK[:], 2, 32, 32)
                    c2v = hv(rt[:, 0:64], 2, 0, 32)
                    s2v = hv(rt[:, 256:320], 2, 0, 32)
                    ka = tp.tile([128, 64], fp32, tag="ka")
                    kb = tp.tile([128, 64], fp32, tag="kb")
                    kav = hv(ka[:], 2, 0, 32)
                    kbv = hv(kb[:], 2, 0, 32)
                    nc.vector.tensor_tensor(out=kav, in0=k1, in1=c2v, op=MUL)
                    nc.vector.tensor_tensor(out=kbv, in0=k2, in1=s2v, op=MUL)
                    nc.vector.tensor_tensor(out=hv(Ksb[:], 2, 0, 32), in0=kav, in1=kbv, op=SUB)
                    kc_ = tp.tile([128, 64], fp32, tag="kc")
                    kd_ = tp.tile([128, 64], fp32, tag="kd")
                    kcv = hv(kc_[:], 2, 0, 32)
                    kdv = hv(kd_[:], 2, 0, 32)
                    nc.vector.tensor_tensor(out=kcv, in0=k2, in1=c2v, op=MUL)
                    nc.vector.tensor_tensor(out=kdv, in0=k1, in1=s2v, op=MUL)
                    nc.vector.tensor_tensor(out=hv(Ksb[:], 2, 32, 32), in0=kcv, in1=kdv, op=ADD)

                    # V: [V_kv0 | 1 | V_kv1 | 1]
                    nc.vector.tensor_copy(Vs[t_i][:, 0:64], psV[:, 0:64])
                    nc.vector.tensor_copy(Vs[t_i][:, 65:129], psV[:, 64:128])
                    nc.gpsimd.memset(Vs[t_i][:, 64:65], 1.0)
                    nc.gpsimd.memset(Vs[t_i][:, 129:130], 1.0)

                    # transposes: Q -> QT (4), K -> KTd (1, duplicated rows)
                    for q in range(4):
                        pt = pst.tile([128, 128], bf16, tag="pt")
                        nc.tensor.transpose(pt[:], Qsb[:, q * 128:(q + 1) * 128], iden[:])
                        nc.vector.tensor_copy(QT[q][:, ts], pt[:])
                    pt2 = pst.tile([128, 128], bf16, tag="pt")
                    nc.tensor.transpose(pt2[:], Ksb[:], iden[:])
                    nc.vector.tensor_copy(KTd[0][0:64, ts], pt2[0:64, :])
                    nc.gpsimd.tensor_copy(KTd[0][64:128, ts], KTd[0][0:64, ts])
                    nc.vector.tensor_copy(KTd[1][0:64, ts], pt2[64:128, :])
                    nc.gpsimd.tensor_copy(KTd[1][64:128, ts], KTd[1][0:64, ts])

            # ---------------- Phase 2: attention ------------------------------
            with (
                tc.tile_pool(name="psS", bufs=2, space="PSUM") as psSp,
                tc.tile_pool(name="psO", bufs=1, space="PSUM") as psOp,
                tc.tile_pool(name="sa", bufs=6) as sap,
                tc.tile_pool(name="fin", bufs=4) as finp,
            ):
                for hp in range(4):
                    kv = hp // 2
                    for isc in range(4):
                        psOT = [psOp.tile([65, 512], fp32, tag=f"psOT{h2}", name=f"psOT{h2}") for h2 in range(2)]
                        njc = 4 * isc + 4
                        for jc in range(njc):
                            r = jc - 4 * isc
                            col0 = max(0, r * 128)
                            js = slice(jc * 128, (jc + 1) * 128)
                            isl = slice(isc * 512 + col0, (isc + 1) * 512)
                            psS = [psSp.tile([128, 512], fp32, tag=f"psS{h2}", name=f"psS{h2}") for h2 in range(2)]
                            SA = [sap.tile([128, 512], bf16, tag=f"SA{h2}", name=f"SA{h2}") for h2 in range(2)]
                            for h2 in range(2):
                                prow = slice(64 * h2, 64 * h2 + 64)
                                nc.tensor.matmul(
                                    psS[h2][:, col0:512],
                                    KTd[kv][prow, js],
                                    QT[hp][prow, isl],
                                    start=True, stop=True,
                                    tile_position=(64 * h2, 0),
                                )
                                nc.scalar.activation(
                                    SA[h2][:, col0:512], psS[h2][:, col0:512],
                                    EXP, scale=0.125,
                                )
                                if r >= 0:
                                    nc.vector.tensor_tensor(
                                        out=SA[h2][:, col0:col0 + 128],
                                        in0=SA[h2][:, col0:col0 + 128],
                                        in1=mask[:], op=MUL,
                                    )
                                nc.tensor.matmul(
                                    psOT[h2][:, col0:512],
                                    Vs[jc][:, 65 * kv:65 * kv + 65],
                                    SA[h2][:, col0:512],
                                    start=(jc == 0), stop=(jc == njc - 1),
                                )
                        for h2 in range(2):
                            rz = finp.tile([1, 512], fp32, tag="rz")
                            nc.vector.reciprocal(rz[:], psOT[h2][64:65, :])
                            bz = finp.tile([64, 512], fp32, tag="bz")
                            nc.gpsimd.partition_broadcast(bz[:], rz[:])
                            nc.vector.tensor_tensor(
                                out=yT[hp][64 * h2:64 * h2 + 64, isc * 512:(isc + 1) * 512],
                                in0=psOT[h2][0:64, :], in1=bz[:], op=MUL,
                            )

            # ---------------- Phase 3: output projection ----------------------
            with (
                tc.tile_pool(name="wo", bufs=1) as wop,
                tc.tile_pool(name="psF", bufs=4, space="PSUM") as psFp,
                tc.tile_pool(name="osb", bufs=4) as osbp,
            ):
                wos = []
                for kc in range(4):
                    wo = wop.tile([128, T], bf16, tag=f"wo{kc}")
                    nc.sync.dma_start(wo[:], wo_d[kc * 128:(kc + 1) * 128, :])
                    wos.append(wo)
                for t_i in range(NT):
                    ts = slice(t_i * 128, (t_i + 1) * 128)
                    for ec in range(4):
                        es = slice(ec * 512, (ec + 1) * 512)
                        psF = psFp.tile([128, 512], fp32, tag="psF")
                        for kc in range(4):
                            nc.tensor.matmul(
                                psF[:], yT[kc][:, ts], wos[kc][:, es],
                                start=(kc == 0), stop=(kc == 3),
                            )
                        ot = osbp.tile([128, 512], fp32, tag="ot")
                        nc.vector.tensor_copy(ot[:], psF[:])
                        nc.sync.dma_start(out_d[ts, es], ot[:])

    nc.compile()
    return nc


def _host_prep(inputs):
    x = np.asarray(inputs["x"], np.float32)
    Wq = np.asarray(inputs["Wq"], np.float32)
    bq = np.asarray(inputs["bq"], np.float32)
    Wk = np.asarray(inputs["Wk"], np.float32)
    bk = np.asarray(inputs["bk"], np.float32)
    Wv = np.asarray(inputs["Wv"], np.float32)
    bv = np.asarray(inputs["bv"], np.float32)
    Wo = np.asarray(inputs["Wo"], np.float32)

    pos = np.arange(1, T + 1, dtype=np.float32)[:, None]
    freqs = 10000.0 ** (-(2.0 * np.arange(D // 2, dtype=np.float32)) / D)
    theta = pos * freqs
    cos_t = np.cos(theta).astype(np.float32)
    sin_t = np.sin(theta).astype(np.float32)
    ropeT = np.ascontiguousarray(np.concatenate(
        [np.tile(cos_t, (1, 8)), np.tile(sin_t, (1, 8))], axis=1))
    mask = (np.arange(128)[:, None] <= np.arange(128)[None, :]).astype(BF16)
    iden = np.eye(128, dtype=BF16)

    xT_aug = np.zeros((B, KAUG, T), np.float32)
    for b in range(B):
        xT_aug[b, :E] = x[b].T
        xT_aug[b, E] = 1.0
    xT_aug = xT_aug.astype(BF16)

    in_maps = []
    for c in range(8):
        b, p = c // 4, c % 4
        wq_a = np.zeros((KAUG, 512), np.float32)
        wq_a[:E] = Wq[:, 512 * p:512 * (p + 1)]
        wq_a[E] = bq[512 * p:512 * (p + 1)]
        wk_a = np.zeros((KAUG, 128), np.float32)
        wk_a[:E] = Wk[:, 128 * p:128 * (p + 1)]
        wk_a[E] = bk[128 * p:128 * (p + 1)]
        wv_a = np.zeros((KAUG, 128), np.float32)
        wv_a[:E] = Wv[:, 128 * p:128 * (p + 1)]
        wv_a[E] = bv[128 * p:128 * (p + 1)]
        in_maps.append({
            "xt": xT_aug[b],
            "wq": wq_a.astype(BF16),
            "wk": wk_a.astype(BF16),
            "wv": wv_a.astype(BF16),
            "wo": np.ascontiguousarray(Wo[512 * p:512 * (p + 1), :]).astype(BF16),
            "rope": ropeT,
            "mask": mask, "iden": iden,
        })
    return in_maps


def _run(inputs, trace=False):
    from concourse.bass_utils import run_bass_kernel_spmd

    if "nc" not in _cache:
        _cache["nc"] = _build_program()
    nc = _cache["nc"]
    in_maps = _host_prep(inputs)
    res = run_bass_kernel_spmd(nc, in_maps, core_ids=list(range(8)), trace=trace)
    bo = np.asarray(inputs["bo"], np.float32)
    out = np.zeros((B, T, E), np.float32)
    for b in range(B):
        acc = bo[None, :].repeat(T, 0).astype(np.float32)
        for c in range(4 * b, 4 * b + 4):
            acc = acc + res.results[c]["out"]
        out[b] = acc
    return out, res


def kernel(**inputs):
    out, _ = _run(inputs, trace=False)
    return out


# revision 10
# speedup vs baseline: 1.1144x; 1.1144x over previous
"""GQA forward on 8 Trainium2 NeuronCores.

Sharding: core c -> batch b=c//4, kv-head pair p=c%4 (kv heads {2p,2p+1},
q heads 8p..8p+7). Each core computes a partial [T,E] output (its heads'
contribution through Wo rows); host sums the 4 partials per batch + bo.

Device layout choices:
- Host passes x[b].T (bias-augmented, bf16) so no on-device transpose of x.
- Q/K/V projections computed in [t,d] layout; RoPE applied there with
  host-precomputed t-major cos/sin tables; Q,K then PE-transposed to [d,t].
- Scores computed as S^T = K Q^T (row-packed matmul pairs, K=64 each),
  exp on ScalarE (scale=1/8 folded in), causal handled by block skipping +
  one [128,128] mask multiply per diagonal block.
- A@V computed in O^T form: lhsT = [V | ones] (stationary), rhs = exp(S^T),
  giving [65, i] PSUM tiles where row 64 is the softmax denominator Z.
  Normalize via reciprocal + gpsimd partition_broadcast + fused DVE multiply
  straight into the yT buffer, which is already the lhsT layout needed for
  the output projection.
"""
import sys
import numpy as np

sys.path.insert(0, "/opt/trn_rl_repo")

import ml_dtypes

BF16 = ml_dtypes.bfloat16

B, T, E = 2, 2048, 2048
HQ, HKV = 32, 8
D = 64
NT = T // 128          # 16 t-chunks
KC = 17                # augmented contraction chunks (2048 + bias row -> 2176)
KAUG = KC * 128

_cache = {}


def _build_program():
    import concourse.bass as bass
    import concourse.tile as tile
    import concourse.mybir as mybir
    from concourse import bacc

    fp32 = mybir.dt.float32
    bf16 = mybir.dt.bfloat16
    MUL = mybir.AluOpType.mult
    ADD = mybir.AluOpType.add
    SUB = mybir.AluOpType.subtract
    EXP = mybir.ActivationFunctionType.Exp

    nc = bacc.Bacc("TRN2", target_bir_lowering=False, debug=False)

    xt_d = nc.dram_tensor("xt", [KAUG, T], bf16, kind="ExternalInput").ap()
    wq_d = nc.dram_tensor("wq", [KAUG, 512], bf16, kind="ExternalInput").ap()
    wk_d = nc.dram_tensor("wk", [KAUG, 128], bf16, kind="ExternalInput").ap()
    wv_d = nc.dram_tensor("wv", [KAUG, 128], bf16, kind="ExternalInput").ap()
    wo_d = nc.dram_tensor("wo", [512, T], bf16, kind="ExternalInput").ap()
    rope_d = nc.dram_tensor("rope", [T, 512], fp32, kind="ExternalInput").ap()
    mask_d = nc.dram_tensor("mask", [128, 128], bf16, kind="ExternalInput").ap()
    iden_d = nc.dram_tensor("iden", [128, 128], bf16, kind="ExternalInput").ap()
    out_d = nc.dram_tensor("out", [T, E], fp32, kind="ExternalOutput").ap()

    def hv(ap, H, off, w):
        # [128, H*64] -> [128, H, w] slice of each head's d-range [off, off+w)
        return ap.rearrange("p (h d) -> p h d", h=H)[:, :, off:off + w]

    with tile.TileContext(nc) as tc:
        with (
            tc.tile_pool(name="persist", bufs=1) as pp,
            tc.tile_pool(name="xpool", bufs=1) as xp,
            tc.tile_pool(name="wpool", bufs=1) as wp,
        ):
            iden = pp.tile([128, 128], bf16)
            nc.sync.dma_start(iden[:], iden_d[:])
            mask = pp.tile([128, 128], bf16)
            nc.sync.dma_start(mask[:], mask_d[:])

            xts = []
            wqs, wks, wvs = [], [], []
            for kc in range(KC):
                xt = xp.tile([128, T], bf16, tag=f"xt{kc}")
                nc.sync.dma_start(xt[:], xt_d[kc * 128:(kc + 1) * 128, :])
                xts.append(xt)
                wq = wp.tile([128, 512], bf16, tag=f"wq{kc}")
                nc.sync.dma_start(wq[:], wq_d[kc * 128:(kc + 1) * 128, :])
                wqs.append(wq)
                wk = wp.tile([128, 128], bf16, tag=f"wk{kc}")
                nc.sync.dma_start(wk[:], wk_d[kc * 128:(kc + 1) * 128, :])
                wks.append(wk)
                wv = wp.tile([128, 128], bf16, tag=f"wv{kc}")
                nc.sync.dma_start(wv[:], wv_d[kc * 128:(kc + 1) * 128, :])
                wvs.append(wv)

            # persistent intermediate buffers
            QT = [pp.tile([128, T], bf16, tag=f"QT{i}", name=f"QT{i}") for i in range(4)]
            KTd = [pp.tile([128, T], bf16, tag=f"KTd{i}", name=f"KTd{i}") for i in range(2)]
            Vs = [pp.tile([128, 130], bf16, tag=f"V{j}", name=f"V{j}") for j in range(NT)]
            yT = [pp.tile([128, T], bf16, tag=f"yT{i}", name=f"yT{i}") for i in range(4)]

            # ---------------- Phase 1: projections + rope + transposes --------
            with (
                tc.tile_pool(name="ps1", bufs=2, space="PSUM") as ps1,
                tc.tile_pool(name="pskv", bufs=2, space="PSUM") as pskv,
                tc.tile_pool(name="pst", bufs=2, space="PSUM") as pst,
                tc.tile_pool(name="rope", bufs=2) as rp,
                tc.tile_pool(name="tmp", bufs=4) as tp,
                tc.tile_pool(name="qk", bufs=3) as qkp,
            ):
                pend = []   # pipelined transposes: (Qsb, Ksb, t-slice)

                def emit_transposes():
                    Qsb_p, Ksb_p, ts_p = pend.pop(0)
                    for q in range(4):
                        pt = pst.tile([128, 128], bf16, tag="pt", name="pt")
                        nc.tensor.transpose(pt[:], Qsb_p[:, q * 128:(q + 1) * 128], iden[:])
                        nc.vector.tensor_copy(QT[q][:, ts_p], pt[:])
                    pt2 = pst.tile([128, 128], bf16, tag="pt", name="pt2")
                    nc.tensor.transpose(pt2[:], Ksb_p[:], iden[:])
                    nc.vector.tensor_copy(KTd[0][0:64, ts_p], pt2[0:64, :])
                    nc.gpsimd.tensor_copy(KTd[0][64:128, ts_p], KTd[0][0:64, ts_p])
                    nc.vector.tensor_copy(KTd[1][0:64, ts_p], pt2[64:128, :])
                    nc.gpsimd.tensor_copy(KTd[1][64:128, ts_p], KTd[1][0:64, ts_p])

                for t_i in range(NT):
                    ts = slice(t_i * 128, (t_i + 1) * 128)
                    psQ = ps1.tile([128, 512], fp32, tag="psQ")
                    psK_t = pskv.tile([128, 128], fp32, tag="psK", name="psK")
                    psV_t = pskv.tile([128, 128], fp32, tag="psV", name="psV")
                    psK = psK_t[:]
                    psV = psV_t[:]
                    for kc in range(KC):
                        st, sp = kc == 0, kc == KC - 1
                        lhs = xts[kc][:, ts]
                        nc.tensor.matmul(psQ[:], lhs, wqs[kc][:], start=st, stop=sp)
                        nc.tensor.matmul(psK, lhs, wks[kc][:], start=st, stop=sp)
                        nc.tensor.matmul(psV, lhs, wvs[kc][:], start=st, stop=sp)

                    rt = rp.tile([128, 512], fp32, tag="rt")
                    nc.sync.dma_start(rt[:], rope_d[ts, :])

                    # RoPE Q on DVE: y1' = y1*c - y2*s ; y2' = y2*c + y1*s
                    Qsb = qkp.tile([128, 512], bf16, tag="Qsb")
                    q1 = hv(psQ[:], 8, 0, 32)
                    q2 = hv(psQ[:], 8, 32, 32)
                    c8v = hv(rt[:, 0:256], 8, 0, 32)
                    s8v = hv(rt[:, 256:512], 8, 0, 32)
                    ta = tp.tile([128, 256], fp32, tag="ta")
                    tb = tp.tile([128, 256], fp32, tag="tb")
                    tav = hv(ta[:], 8, 0, 32)
                    tbv = hv(tb[:], 8, 0, 32)
                    nc.vector.tensor_tensor(out=tav, in0=q1, in1=c8v, op=MUL)
                    nc.vector.tensor_tensor(out=tbv, in0=q2, in1=s8v, op=MUL)
                    nc.vector.tensor_tensor(out=hv(Qsb[:], 8, 0, 32), in0=tav, in1=tbv, op=SUB)
                    tc_ = tp.tile([128, 256], fp32, tag="tc")
                    td_ = tp.tile([128, 256], fp32, tag="td")
                    tcv = hv(tc_[:], 8, 0, 32)
                    tdv = hv(td_[:], 8, 0, 32)
                    nc.vector.tensor_tensor(out=tcv, in0=q2, in1=c8v, op=MUL)
                    nc.vector.tensor_tensor(out=tdv, in0=q1, in1=s8v, op=MUL)
                    nc.vector.tensor_tensor(out=hv(Qsb[:], 8, 32, 32), in0=tcv, in1=tdv, op=ADD)

                    # RoPE K on GpSimd
                    Ksb = qkp.tile([128, 128], bf16, tag="Ksb")
                    k1 = hv(psK, 2, 0, 32)
                    k2 = hv(psK, 2, 32, 32)
                    c2v = hv(rt[:, 0:64], 2, 0, 32)
                    s2v = hv(rt[:, 256:320], 2, 0, 32)
                    ka = tp.tile([128, 64], fp32, tag="ka")
                    kb = tp.tile([128, 64], fp32, tag="kb")
                    kav = hv(ka[:], 2, 0, 32)
                    kbv = hv(kb[:], 2, 0, 32)
                    nc.vector.tensor_tensor(out=kav, in0=k1, in1=c2v, op=MUL)
                    nc.vector.tensor_tensor(out=kbv, in0=k2, in1=s2v, op=MUL)
                    nc.vector.tensor_tensor(out=hv(Ksb[:], 2, 0, 32), in0=kav, in1=kbv, op=SUB)
                    kc_ = tp.tile([128, 64], fp32, tag="kc")
                    kd_ = tp.tile([128, 64], fp32, tag="kd")
                    kcv = hv(kc_[:], 2, 0, 32)
                    kdv = hv(kd_[:], 2, 0, 32)
                    nc.vector.tensor_tensor(out=kcv, in0=k2, in1=c2v, op=MUL)
                    nc.vector.tensor_tensor(out=kdv, in0=k1, in1=s2v, op=MUL)
                    nc.vector.tensor_tensor(out=hv(Ksb[:], 2, 32, 32), in0=kcv, in1=kdv, op=ADD)

                    # V: [V_kv0 | 1 | V_kv1 | 1]
                    nc.vector.tensor_copy(Vs[t_i][:, 0:64], psV[:, 0:64])
                    nc.vector.tensor_copy(Vs[t_i][:, 65:129], psV[:, 64:128])
                    nc.gpsimd.memset(Vs[t_i][:, 64:65], 1.0)
                    nc.gpsimd.memset(Vs[t_i][:, 129:130], 1.0)

                    # pipeline: transpose previous chunk's Q/K after this chunk's MMs
                    pend.append((Qsb, Ksb, ts))
                    if len(pend) > 1:
                        emit_transposes()
                for _ in range(len(pend)):
                    emit_transposes()

            # ---------------- Phase 2: attention ------------------------------
            with (
                tc.tile_pool(name="psS", bufs=2, space="PSUM") as psSp,
                tc.tile_pool(name="psO", bufs=2, space="PSUM") as psOp,
                tc.tile_pool(name="sa", bufs=6) as sap,
                tc.tile_pool(name="fin", bufs=4) as finp,
            ):
                for hp in range(4):
                    kv = hp // 2
                    for isc in range(4):
                        psOT = [psOp.tile([65, 512], fp32, tag=f"psOT{h2}", name=f"psOT{h2}") for h2 in range(2)]
                        njc = 4 * isc + 4

                        def emit_S(jc):
                            r = jc - 4 * isc
                            col0 = max(0, r * 128)
                            js = slice(jc * 128, (jc + 1) * 128)
                            isl = slice(isc * 512 + col0, (isc + 1) * 512)
                            SA = [sap.tile([128, 512], bf16, tag=f"SA{h2}", name=f"SA{h2}") for h2 in range(2)]
                            for h2 in range(2):
                                prow = slice(64 * h2, 64 * h2 + 64)
                                psS = psSp.tile([128, 512], fp32, tag=f"psS{h2}", name=f"psS{h2}")
                                nc.tensor.matmul(
                                    psS[:, col0:512],
                                    KTd[kv][prow, js],
                                    QT[hp][prow, isl],
                                    start=True, stop=True,
                                    tile_position=(64 * h2, 0),
                                )
                                nc.scalar.activation(
                                    SA[h2][:, col0:512], psS[:, col0:512],
                                    EXP, scale=0.125,
                                )
                                if r >= 0:
                                    nc.vector.tensor_tensor(
                                        out=SA[h2][:, col0:col0 + 128],
                                        in0=SA[h2][:, col0:col0 + 128],
                                        in1=mask[:], op=MUL,
                                    )
                            return SA, col0

                        ready = emit_S(0)
                        for jc in range(njc):
                            SA, col0 = ready
                            if jc + 1 < njc:
                                ready = emit_S(jc + 1)
                            for h2 in range(2):
                                nc.tensor.matmul(
                                    psOT[h2][:, col0:512],
                                    Vs[jc][:, 65 * kv:65 * kv + 65],
                                    SA[h2][:, col0:512],
                                    start=(jc == 0), stop=(jc == njc - 1),
                                )
                        for h2 in range(2):
                            rz = finp.tile([1, 512], fp32, tag="rz")
                            nc.vector.reciprocal(rz[:], psOT[h2][64:65, :])
                            bz = finp.tile([64, 512], fp32, tag="bz")
                            nc.gpsimd.partition_broadcast(bz[:], rz[:])
                            nc.vector.tensor_tensor(
                                out=yT[hp][64 * h2:64 * h2 + 64, isc * 512:(isc + 1) * 512],
                                in0=psOT[h2][0:64, :], in1=bz[:], op=MUL,
                            )

            # ---------------- Phase 3: output projection ----------------------
            with (
                tc.tile_pool(name="wo", bufs=1) as wop,
                tc.tile_pool(name="psF", bufs=4, space="PSUM") as psFp,
                tc.tile_pool(name="osb", bufs=4) as osbp,
            ):
                wos = []
                for kc in range(4):
                    wo = wop.tile([128, T], bf16, tag=f"wo{kc}")
                    nc.sync.dma_start(wo[:], wo_d[kc * 128:(kc + 1) * 128, :])
                    wos.append(wo)
                for t_i in range(NT):
                    ts = slice(t_i * 128, (t_i + 1) * 128)
                    for ec in range(4):
                        es = slice(ec * 512, (ec + 1) * 512)
                        psF = psFp.tile([128, 512], fp32, tag="psF")
                        for kc in range(4):
                            nc.tensor.matmul(
                                psF[:], yT[kc][:, ts], wos[kc][:, es],
                                start=(kc == 0), stop=(kc == 3),
                            )
                        ot = osbp.tile([128, 512], fp32, tag="ot")
                        nc.vector.tensor_copy(ot[:], psF[:])
                        nc.sync.dma_start(out_d[ts, es], ot[:])

    nc.compile()
    return nc


def _host_prep(inputs):
    x = np.asarray(inputs["x"], np.float32)
    Wq = np.asarray(inputs["Wq"], np.float32)
    bq = np.asarray(inputs["bq"], np.float32)
    Wk = np.asarray(inputs["Wk"], np.float32)
    bk = np.asarray(inputs["bk"], np.float32)
    Wv = np.asarray(inputs["Wv"], np.float32)
    bv = np.asarray(inputs["bv"], np.float32)
    Wo = np.asarray(inputs["Wo"], np.float32)

    pos = np.arange(1, T + 1, dtype=np.float32)[:, None]
    freqs = 10000.0 ** (-(2.0 * np.arange(D // 2, dtype=np.float32)) / D)
    theta = pos * freqs
    cos_t = np.cos(theta).astype(np.float32)
    sin_t = np.sin(theta).astype(np.float32)
    ropeT = np.ascontiguousarray(np.concatenate(
        [np.tile(cos_t, (1, 8)), np.tile(sin_t, (1, 8))], axis=1))
    mask = (np.arange(128)[:, None] <= np.arange(128)[None, :]).astype(BF16)
    iden = np.eye(128, dtype=BF16)

    xT_aug = np.zeros((B, KAUG, T), np.float32)
    for b in range(B):
        xT_aug[b, :E] = x[b].T
        xT_aug[b, E] = 1.0
    xT_aug = xT_aug.astype(BF16)

    in_maps = []
    for c in range(8):
        b, p = c // 4, c % 4
        wq_a = np.zeros((KAUG, 512), np.float32)
        wq_a[:E] = Wq[:, 512 * p:512 * (p + 1)]
        wq_a[E] = bq[512 * p:512 * (p + 1)]
        wk_a = np.zeros((KAUG, 128), np.float32)
        wk_a[:E] = Wk[:, 128 * p:128 * (p + 1)]
        wk_a[E] = bk[128 * p:128 * (p + 1)]
        wv_a = np.zeros((KAUG, 128), np.float32)
        wv_a[:E] = Wv[:, 128 * p:128 * (p + 1)]
        wv_a[E] = bv[128 * p:128 * (p + 1)]
        in_maps.append({
            "xt": xT_aug[b],
            "wq": wq_a.astype(BF16),
            "wk": wk_a.astype(BF16),
            "wv": wv_a.astype(BF16),
            "wo": np.ascontiguousarray(Wo[512 * p:512 * (p + 1), :]).astype(BF16),
            "rope": ropeT,
            "mask": mask, "iden": iden,
        })
    return in_maps


def _run(inputs, trace=False):
    from concourse.bass_utils import run_bass_kernel_spmd

    if "nc" not in _cache:
        _cache["nc"] = _build_program()
    nc = _cache["nc"]
    in_maps = _host_prep(inputs)
    res = run_bass_kernel_spmd(nc, in_maps, core_ids=list(range(8)), trace=trace)
    bo = np.asarray(inputs["bo"], np.float32)
    out = np.zeros((B, T, E), np.float32)
    for b in range(B):
        acc = bo[None, :].repeat(T, 0).astype(np.float32)
        for c in range(4 * b, 4 * b + 4):
            acc = acc + res.results[c]["out"]
        out[b] = acc
    return out, res


def kernel(**inputs):
    out, _ = _run(inputs, trace=False)
    return out


# revision 12
# speedup vs baseline: 1.4070x; 1.2625x over previous
"""GQA forward on 8 Trainium2 NeuronCores.

Sharding: core c -> batch b=c//4, kv-head pair p=c%4 (kv heads {2p,2p+1},
q heads 8p..8p+7). Each core computes a partial [T,E] output (its heads'
contribution through Wo rows); host sums the 4 partials per batch + bo.

Device layout choices:
- Host passes x[b].T (bias-augmented, bf16) so no on-device transpose of x.
- Q/K/V projections computed in [t,d] layout; RoPE applied there with
  host-precomputed t-major cos/sin tables; Q,K then PE-transposed to [d,t].
- Scores computed as S^T = K Q^T (row-packed matmul pairs, K=64 each),
  exp on ScalarE (scale=1/8 folded in), causal handled by block skipping +
  one [128,128] mask multiply per diagonal block.
- A@V computed in O^T form: lhsT = [V | ones] (stationary), rhs = exp(S^T),
  giving [65, i] PSUM tiles where row 64 is the softmax denominator Z.
  Normalize via reciprocal + gpsimd partition_broadcast + fused DVE multiply
  straight into the yT buffer, which is already the lhsT layout needed for
  the output projection.
"""
import sys
import numpy as np

sys.path.insert(0, "/opt/trn_rl_repo")

import ml_dtypes

BF16 = ml_dtypes.bfloat16

B, T, E = 2, 2048, 2048
HQ, HKV = 32, 8
D = 64
NT = T // 128          # 16 t-chunks
KC = 17                # augmented contraction chunks (2048 + bias row -> 2176)
KAUG = KC * 128

_cache = {}


def _build_program():
    import concourse.bass as bass
    import concourse.tile as tile
    import concourse.mybir as mybir
    from concourse import bacc

    fp32 = mybir.dt.float32
    bf16 = mybir.dt.bfloat16
    MUL = mybir.AluOpType.mult
    ADD = mybir.AluOpType.add
    SUB = mybir.AluOpType.subtract
    EXP = mybir.ActivationFunctionType.Exp

    nc = bacc.Bacc("TRN2", target_bir_lowering=False, debug=False)

    xt_d = nc.dram_tensor("xt", [KAUG, T], bf16, kind="ExternalInput").ap()
    wq_d = nc.dram_tensor("wq", [KAUG, 512], bf16, kind="ExternalInput").ap()
    wk_d = nc.dram_tensor("wk", [KAUG, 128], bf16, kind="ExternalInput").ap()
    wv_d = nc.dram_tensor("wv", [KAUG, 128], bf16, kind="ExternalInput").ap()
    wo_d = nc.dram_tensor("wo", [512, T], bf16, kind="ExternalInput").ap()
    rope_d = nc.dram_tensor("rope", [T, 512], fp32, kind="ExternalInput").ap()
    mask_d = nc.dram_tensor("mask", [128, 128], bf16, kind="ExternalInput").ap()
    iden_d = nc.dram_tensor("iden", [128, 128], bf16, kind="ExternalInput").ap()
    out_d = nc.dram_tensor("out", [T, E], fp32, kind="ExternalOutput").ap()

    def hv(ap, H, off, w):
        # [128, H*64] -> [128, H, w] slice of each head's d-range [off, off+w)
        return ap.rearrange("p (h d) -> p h d", h=H)[:, :, off:off + w]

    with tile.TileContext(nc) as tc:
        with (
            tc.tile_pool(name="persist", bufs=1) as pp,
            tc.tile_pool(name="xpool", bufs=1) as xp,
            tc.tile_pool(name="wpool", bufs=1) as wp,
        ):
            iden = pp.tile([128, 128], bf16)
            nc.sync.dma_start(iden[:], iden_d[:])
            mask = pp.tile([128, 128], bf16)
            nc.sync.dma_start(mask[:], mask_d[:])

            xts = []
            wqs, wks, wvs = [], [], []
            for kc in range(KC):
                xt = xp.tile([128, T], bf16, tag=f"xt{kc}")
                nc.sync.dma_start(xt[:], xt_d[kc * 128:(kc + 1) * 128, :])
                xts.append(xt)
                wq = wp.tile([128, 512], bf16, tag=f"wq{kc}")
                nc.sync.dma_start(wq[:], wq_d[kc * 128:(kc + 1) * 128, :])
                wqs.append(wq)
                wk = wp.tile([128, 128], bf16, tag=f"wk{kc}")
                nc.sync.dma_start(wk[:], wk_d[kc * 128:(kc + 1) * 128, :])
                wks.append(wk)
                wv = wp.tile([128, 128], bf16, tag=f"wv{kc}")
                nc.sync.dma_start(wv[:], wv_d[kc * 128:(kc + 1) * 128, :])
                wvs.append(wv)

            # persistent intermediate buffers
            QT = [pp.tile([128, T], bf16, tag=f"QT{i}", name=f"QT{i}") for i in range(4)]
            KTd = [pp.tile([128, T], bf16, tag=f"KTd{i}", name=f"KTd{i}") for i in range(2)]
            Vs = [pp.tile([128, 256], bf16, tag=f"V{j}", name=f"V{j}") for j in range(NT)]
            yT = [pp.tile([128, T], bf16, tag=f"yT{i}", name=f"yT{i}") for i in range(4)]

            # ---------------- Phase 1: projections + rope + transposes --------
            with (
                tc.tile_pool(name="ps1", bufs=2, space="PSUM") as ps1,
                tc.tile_pool(name="pskv", bufs=2, space="PSUM") as pskv,
                tc.tile_pool(name="pst", bufs=2, space="PSUM") as pst,
                tc.tile_pool(name="rope", bufs=2) as rp,
                tc.tile_pool(name="tmp", bufs=4) as tp,
                tc.tile_pool(name="qk", bufs=3) as qkp,
            ):
                pend = []   # pipelined transposes: (Qsb, Ksb, t-slice)

                def emit_transposes():
                    Qsb_p, Ksb_p, ts_p = pend.pop(0)
                    for q in range(4):
                        pt = pst.tile([128, 128], bf16, tag="pt", name="pt")
                        nc.tensor.transpose(pt[:], Qsb_p[:, q * 128:(q + 1) * 128], iden[:])
                        nc.scalar.copy(QT[q][:, ts_p], pt[:])
                    pt2 = pst.tile([128, 128], bf16, tag="pt", name="pt2")
                    nc.tensor.transpose(pt2[:], Ksb_p[:], iden[:])
                    nc.scalar.copy(KTd[0][0:64, ts_p], pt2[0:64, :])
                    nc.gpsimd.tensor_copy(KTd[0][64:128, ts_p], KTd[0][0:64, ts_p])
                    nc.scalar.copy(KTd[1][0:64, ts_p], pt2[64:128, :])
                    nc.gpsimd.tensor_copy(KTd[1][64:128, ts_p], KTd[1][0:64, ts_p])

                for t_i in range(NT):
                    ts = slice(t_i * 128, (t_i + 1) * 128)
                    psQ = ps1.tile([128, 512], fp32, tag="psQ")
                    psK_t = pskv.tile([128, 128], fp32, tag="psK", name="psK")
                    psV_t = pskv.tile([128, 128], fp32, tag="psV", name="psV")
                    psK = psK_t[:]
                    psV = psV_t[:]
                    for kc in range(KC):
                        st, sp = kc == 0, kc == KC - 1
                        lhs = xts[kc][:, ts]
                        nc.tensor.matmul(psQ[:], lhs, wqs[kc][:], start=st, stop=sp)
                        nc.tensor.matmul(psK, lhs, wks[kc][:], start=st, stop=sp)
                        nc.tensor.matmul(psV, lhs, wvs[kc][:], start=st, stop=sp)

                    rt = rp.tile([128, 512], fp32, tag="rt")
                    nc.sync.dma_start(rt[:], rope_d[ts, :])

                    # RoPE Q on DVE: y1' = y1*c - y2*s ; y2' = y2*c + y1*s
                    Qsb = qkp.tile([128, 512], bf16, tag="Qsb")
                    q1 = hv(psQ[:], 8, 0, 32)
                    q2 = hv(psQ[:], 8, 32, 32)
                    c8v = hv(rt[:, 0:256], 8, 0, 32)
                    s8v = hv(rt[:, 256:512], 8, 0, 32)
                    ta = tp.tile([128, 256], fp32, tag="ta")
                    tb = tp.tile([128, 256], fp32, tag="tb")
                    tav = hv(ta[:], 8, 0, 32)
                    tbv = hv(tb[:], 8, 0, 32)
                    nc.vector.tensor_tensor(out=tav, in0=q1, in1=c8v, op=MUL)
                    nc.vector.tensor_tensor(out=tbv, in0=q2, in1=s8v, op=MUL)
                    nc.vector.tensor_tensor(out=hv(Qsb[:], 8, 0, 32), in0=tav, in1=tbv, op=SUB)
                    tc_ = tp.tile([128, 256], fp32, tag="tc")
                    td_ = tp.tile([128, 256], fp32, tag="td")
                    tcv = hv(tc_[:], 8, 0, 32)
                    tdv = hv(td_[:], 8, 0, 32)
                    nc.vector.tensor_tensor(out=tcv, in0=q2, in1=c8v, op=MUL)
                    nc.vector.tensor_tensor(out=tdv, in0=q1, in1=s8v, op=MUL)
                    nc.vector.tensor_tensor(out=hv(Qsb[:], 8, 32, 32), in0=tcv, in1=tdv, op=ADD)

                    # RoPE K on GpSimd
                    Ksb = qkp.tile([128, 128], bf16, tag="Ksb")
                    k1 = hv(psK, 2, 0, 32)
                    k2 = hv(psK, 2, 32, 32)
                    c2v = hv(rt[:, 0:64], 2, 0, 32)
                    s2v = hv(rt[:, 256:320], 2, 0, 32)
                    ka = tp.tile([128, 64], fp32, tag="ka")
                    kb = tp.tile([128, 64], fp32, tag="kb")
                    kav = hv(ka[:], 2, 0, 32)
                    kbv = hv(kb[:], 2, 0, 32)
                    nc.vector.tensor_tensor(out=kav, in0=k1, in1=c2v, op=MUL)
                    nc.vector.tensor_tensor(out=kbv, in0=k2, in1=s2v, op=MUL)
                    nc.vector.tensor_tensor(out=hv(Ksb[:], 2, 0, 32), in0=kav, in1=kbv, op=SUB)
                    kc_ = tp.tile([128, 64], fp32, tag="kc")
                    kd_ = tp.tile([128, 64], fp32, tag="kd")
                    kcv = hv(kc_[:], 2, 0, 32)
                    kdv = hv(kd_[:], 2, 0, 32)
                    nc.vector.tensor_tensor(out=kcv, in0=k2, in1=c2v, op=MUL)
                    nc.vector.tensor_tensor(out=kdv, in0=k1, in1=s2v, op=MUL)
                    nc.vector.tensor_tensor(out=hv(Ksb[:], 2, 32, 32), in0=kcv, in1=kdv, op=ADD)

                    # V: [V_kv0 | ones64 | V_kv1 | ones64] -> Z replicated on 64 partitions
                    nc.vector.tensor_copy(Vs[t_i][:, 0:64], psV[:, 0:64])
                    nc.vector.tensor_copy(Vs[t_i][:, 128:192], psV[:, 64:128])
                    nc.gpsimd.memset(Vs[t_i][:, 64:128], 1.0)
                    nc.gpsimd.memset(Vs[t_i][:, 192:256], 1.0)

                    # pipeline: transpose previous chunk's Q/K after this chunk's MMs
                    pend.append((Qsb, Ksb, ts))
                    if len(pend) > 1:
                        emit_transposes()
                for _ in range(len(pend)):
                    emit_transposes()

            # ---------------- Phase 2: attention ------------------------------
            with (
                tc.tile_pool(name="psS", bufs=2, space="PSUM") as psSp,
                tc.tile_pool(name="psO", bufs=2, space="PSUM") as psOp,
                tc.tile_pool(name="sa", bufs=4) as sap,
                tc.tile_pool(name="fin", bufs=4) as finp,
            ):
                for hp in range(4):
                    kv = hp // 2
                    for isc in range(4):
                        psOT = [psOp.tile([128, 512], fp32, tag=f"psOT{h2}", name=f"psOT{h2}") for h2 in range(2)]
                        njc = 4 * isc + 4

                        def emit_S(jc):
                            r = jc - 4 * isc
                            col0 = max(0, r * 128)
                            js = slice(jc * 128, (jc + 1) * 128)
                            isl = slice(isc * 512 + col0, (isc + 1) * 512)
                            SA = sap.tile([128, 1024], bf16, tag="SA", name="SA")
                            psS = psSp.tile([128, 1024], fp32, tag="psS", name="psS")
                            for h2 in range(2):
                                prow = slice(64 * h2, 64 * h2 + 64)
                                nc.tensor.matmul(
                                    psS[:, 512 * h2 + col0:512 * h2 + 512],
                                    KTd[kv][prow, js],
                                    QT[hp][prow, isl],
                                    start=True, stop=True,
                                    tile_position=(64 * h2, 0),
                                )
                            if r < 0:
                                nc.scalar.activation(SA[:], psS[:], EXP, scale=0.125)
                            else:
                                for h2 in range(2):
                                    c = 512 * h2 + col0
                                    nc.scalar.activation(
                                        SA[:, c:512 * h2 + 512], psS[:, c:512 * h2 + 512],
                                        EXP, scale=0.125,
                                    )
                                    nc.vector.tensor_tensor(
                                        out=SA[:, c:c + 128], in0=SA[:, c:c + 128],
                                        in1=mask[:], op=MUL,
                                    )
                            return SA, col0

                        ready = emit_S(0)
                        for jc in range(njc):
                            SA, col0 = ready
                            if jc + 1 < njc:
                                ready = emit_S(jc + 1)
                            for h2 in range(2):
                                nc.tensor.matmul(
                                    psOT[h2][:, col0:512],
                                    Vs[jc][:, 128 * kv:128 * kv + 128],
                                    SA[:, 512 * h2 + col0:512 * h2 + 512],
                                    start=(jc == 0), stop=(jc == njc - 1),
                                )
                        for h2 in range(2):
                            rec = finp.tile([64, 512], fp32, tag="rec", name="rec")
                            nc.vector.reciprocal(rec[:], psOT[h2][64:128, :])
                            nc.vector.tensor_tensor(
                                out=yT[hp][64 * h2:64 * h2 + 64, isc * 512:(isc + 1) * 512],
                                in0=psOT[h2][0:64, :], in1=rec[:], op=MUL,
                            )

            # ---------------- Phase 3: output projection ----------------------
            with (
                tc.tile_pool(name="wo", bufs=1) as wop,
                tc.tile_pool(name="psF", bufs=4, space="PSUM") as psFp,
                tc.tile_pool(name="osb", bufs=4) as osbp,
            ):
                wos = []
                for kc in range(4):
                    wo = wop.tile([128, T], bf16, tag=f"wo{kc}")
                    nc.sync.dma_start(wo[:], wo_d[kc * 128:(kc + 1) * 128, :])
                    wos.append(wo)
                for t_i in range(NT):
                    ts = slice(t_i * 128, (t_i + 1) * 128)
                    for ec in range(4):
                        es = slice(ec * 512, (ec + 1) * 512)
                        psF = psFp.tile([128, 512], fp32, tag="psF")
                        for kc in range(4):
                            nc.tensor.matmul(
                                psF[:], yT[kc][:, ts], wos[kc][:, es],
                                start=(kc == 0), stop=(kc == 3),
                            )
                        ot = osbp.tile([128, 512], fp32, tag="ot")
                        if ec % 2 == 0:
                            nc.vector.tensor_copy(ot[:], psF[:])
                        else:
                            nc.scalar.copy(ot[:], psF[:])
                        nc.sync.dma_start(out_d[ts, es], ot[:])

    nc.compile()
    return nc


def _host_prep(inputs):
    x = np.asarray(inputs["x"], np.float32)
    Wq = np.asarray(inputs["Wq"], np.float32)
    bq = np.asarray(inputs["bq"], np.float32)
    Wk = np.asarray(inputs["Wk"], np.float32)
    bk = np.asarray(inputs["bk"], np.float32)
    Wv = np.asarray(inputs["Wv"], np.float32)
    bv = np.asarray(inputs["bv"], np.float32)
    Wo = np.asarray(inputs["Wo"], np.float32)

    pos = np.arange(1, T + 1, dtype=np.float32)[:, None]
    freqs = 10000.0 ** (-(2.0 * np.arange(D // 2, dtype=np.float32)) / D)
    theta = pos * freqs
    cos_t = np.cos(theta).astype(np.float32)
    sin_t = np.sin(theta).astype(np.float32)
    ropeT = np.ascontiguousarray(np.concatenate(
        [np.tile(cos_t, (1, 8)), np.tile(sin_t, (1, 8))], axis=1))
    mask = (np.arange(128)[:, None] <= np.arange(128)[None, :]).astype(BF16)
    iden = np.eye(128, dtype=BF16)

    xT_aug = np.zeros((B, KAUG, T), np.float32)
    for b in range(B):
        xT_aug[b, :E] = x[b].T
        xT_aug[b, E] = 1.0
    xT_aug = xT_aug.astype(BF16)

    in_maps = []
    for c in range(8):
        b, p = c // 4, c % 4
        wq_a = np.zeros((KAUG, 512), np.float32)
        wq_a[:E] = Wq[:, 512 * p:512 * (p + 1)]
        wq_a[E] = bq[512 * p:512 * (p + 1)]
        wk_a = np.zeros((KAUG, 128), np.float32)
        wk_a[:E] = Wk[:, 128 * p:128 * (p + 1)]
        wk_a[E] = bk[128 * p:128 * (p + 1)]
        wv_a = np.zeros((KAUG, 128), np.float32)
        wv_a[:E] = Wv[:, 128 * p:128 * (p + 1)]
        wv_a[E] = bv[128 * p:128 * (p + 1)]
        in_maps.append({
            "xt": xT_aug[b],
            "wq": wq_a.astype(BF16),
            "wk": wk_a.astype(BF16),
            "wv": wv_a.astype(BF16),
            "wo": np.ascontiguousarray(Wo[512 * p:512 * (p + 1), :]).astype(BF16),
            "rope": ropeT,
            "mask": mask, "iden": iden,
        })
    return in_maps


def _run(inputs, trace=False):
    from concourse.bass_utils import run_bass_kernel_spmd

    if "nc" not in _cache:
        _cache["nc"] = _build_program()
    nc = _cache["nc"]
    in_maps = _host_prep(inputs)
    res = run_bass_kernel_spmd(nc, in_maps, core_ids=list(range(8)), trace=trace)
    bo = np.asarray(inputs["bo"], np.float32)
    out = np.zeros((B, T, E), np.float32)
    for b in range(B):
        acc = bo[None, :].repeat(T, 0).astype(np.float32)
        for c in range(4 * b, 4 * b + 4):
            acc = acc + res.results[c]["out"]
        out[b] = acc
    return out, res


def kernel(**inputs):
    out, _ = _run(inputs, trace=False)
    return out
